# revision 8
# baseline (speedup 1.0000x reference)
"""Trainium2 Bass kernel for nn_AttentionBlock (B=32, C=256, H=W=32).

Computation (per batch element b):
    qkv   = w_in @ x[b] + b_in          # 1x1 conv == channel matmul
    q,k,v = split(qkv)
    S^T   = K^T-style scores, stored transposed: S^T[HW, hw] = sum_c k[c,HW] q[c,hw]
    E     = exp(S^T / 16)               # softmax without max-subtraction (safe range)
    Z[hw] = sum_HW E[HW, hw]            # via ones-vector matmul (partition reduction)
    U^T   = x[b]^T @ (w_out @ w_v)^T    # w_out folded into V-projection (attention is
                                        # linear in V), computed directly transposed
    F     = U^T-contraction: F[o, hw] = sum_HW U^T[HW, o] E[HW, hw]
    out   = F * (1/Z) + (w_out @ b_v + b_out) + x[b]

Sharding: data-parallel over batch, 4 batch elements per core, 8 cores,
weights replicated. All matmul layouts chosen so no on-device transpose
is ever needed.

Env knobs:
    ATTN_MM_DT   = fp32 | f32r | bf16   (matmul operand dtype; default fp32)
    ATTN_BCAST   = dve | dma            (1/Z partition-broadcast path; default dma)
    ATTN_TRACE   = 0 | 1                (collect NTFF trace via run_bass_kernel_spmd)
"""

import math
import os
import sys

import numpy as np

B, C, HW = 32, 256, 1024
NCORES = 8
BPC = B // NCORES  # batch elements per core
CH_T = C // 128  # channel partition tiles (2)
HW_T = HW // 128  # spatial partition tiles (8)
NF = 512  # matmul free-dim chunk (one PSUM bank of fp32)
N_CH = HW // NF  # free-dim chunks (2)

_cache = {}
last_results = None  # BassKernelResults of the most recent run (for test.py)


def _ensure_path():
    for p in ("/opt/trn_rl_repo",):
        if os.path.isdir(p) and p not in sys.path:
            sys.path.append(p)


def _build(mm_dt: str, bcast: str):
    """Build + compile the Bass kernel once. Returns the compiled nc."""
    _ensure_path()
    import concourse.bass as bass
    import concourse.mybir as mybir
    import concourse.tile as tile
    from concourse import bacc

    f32 = mybir.dt.float32
    f32r = mybir.dt.float32r
    bf16 = mybir.dt.bfloat16
    Alu = mybir.AluOpType
    Act = mybir.ActivationFunctionType

    cdt = bf16 if mm_dt == "bf16" else f32  # storage dtype of matmul operands

    def mmv(ap):
        # view an SBUF fp32 AP as float32r for the tensor engine
        return ap.bitcast(f32r) if mm_dt == "f32r" else ap

    nc = bacc.Bacc(
        "TRN2", target_bir_lowering=False, debug=False, enable_asserts=False
    )

    x_d = nc.dram_tensor("x", [BPC, C, HW], f32, kind="ExternalInput")
    wqkT_d = nc.dram_tensor("wqkT", [C, 2 * C], f32, kind="ExternalInput")
    wuT_d = nc.dram_tensor("wuT", [C, C], f32, kind="ExternalInput")
    bqk_d = nc.dram_tensor("bqk", [128, 4], f32, kind="ExternalInput")
    bf_d = nc.dram_tensor("bf", [128, 2], f32, kind="ExternalInput")
    out_d = nc.dram_tensor("out", [BPC, C, HW], f32, kind="ExternalOutput")

    def ns(n):
        return slice(n * NF, (n + 1) * NF)

    with tile.TileContext(nc) as tc:
        with (
            tc.tile_pool(name="const", bufs=1) as const,
            tc.tile_pool(name="xp", bufs=2) as xp,
            tc.tile_pool(name="qkp", bufs=2) as qkp,
            tc.tile_pool(name="up", bufs=2) as up,
            tc.tile_pool(name="ep", bufs=2) as ep,
            tc.tile_pool(name="rp", bufs=2) as rp,
            tc.tile_pool(name="op", bufs=2) as op_,
            tc.tile_pool(name="tp", bufs=4) as tp,
            tc.tile_pool(name="ps", bufs=6, space="PSUM") as ps,
            tc.tile_pool(name="zp", bufs=1, space="PSUM") as zp,
            tc.tile_pool(name="rd", bufs=2, space="DRAM") as rd,
        ):
            def load_x(b):
                x_sb = xp.tile([128, CH_T, HW], f32, tag="x", name="x_sb")
                for ci in range(CH_T):
                    for n in range(N_CH):
                        eng = nc.sync if n == 0 else nc.gpsimd
                        eng.dma_start(
                            out=x_sb[:, ci, ns(n)],
                            in_=x_d[b, ci * 128 : (ci + 1) * 128, ns(n)],
                        )
                return x_sb

            x_next = load_x(0)

            # ---- weights / constants (loaded once) ----
            if mm_dt == "bf16":
                wstg = const.tile([128, CH_T, 2 * C], f32, tag="wstg")
                wustg = const.tile([128, CH_T, C], f32, tag="wustg")
            wqk_sb = const.tile([128, CH_T, 2 * C], cdt, tag="wqk")
            wu_sb = const.tile([128, CH_T, C], cdt, tag="wu")
            for ci in range(CH_T):
                if mm_dt == "bf16":
                    nc.sync.dma_start(
                        out=wstg[:, ci, :], in_=wqkT_d[ci * 128 : (ci + 1) * 128, :]
                    )
                    nc.sync.dma_start(
                        out=wustg[:, ci, :], in_=wuT_d[ci * 128 : (ci + 1) * 128, :]
                    )
                    nc.vector.tensor_copy(wqk_sb[:, ci, :], wstg[:, ci, :])
                    nc.vector.tensor_copy(wu_sb[:, ci, :], wustg[:, ci, :])
                else:
                    nc.sync.dma_start(
                        out=wqk_sb[:, ci, :], in_=wqkT_d[ci * 128 : (ci + 1) * 128, :]
                    )
                    nc.sync.dma_start(
                        out=wu_sb[:, ci, :], in_=wuT_d[ci * 128 : (ci + 1) * 128, :]
                    )
            bqk_sb = const.tile([128, 4], f32, tag="bqk")
            bf_sb = const.tile([128, 2], f32, tag="bf")
            nc.sync.dma_start(out=bqk_sb[:], in_=bqk_d[:])
            nc.sync.dma_start(out=bf_sb[:], in_=bf_d[:])
            ones_col = const.tile([128, 1], cdt, tag="ones")
            nc.vector.memset(ones_col[:], 1.0)

            for b in range(BPC):
                # ---- x[b] as [c_part, hw] (prefetched) ----
                x_sb = x_next
                if b + 1 < BPC:
                    x_next = load_x(b + 1)
                if mm_dt == "bf16":
                    x_cmp = xp.tile([128, CH_T, HW], bf16, tag="xmm")
                    for ci in range(CH_T):
                        for n in range(N_CH):
                            nc.vector.tensor_copy(
                                x_cmp[:, ci, ns(n)], x_sb[:, ci, ns(n)]
                            )
                else:
                    x_cmp = x_sb

                # ---- Q,K projection: qk[m, hw], m = (q0,q1,k0,k1) ----
                qk_sb = qkp.tile([128, 4, HW], cdt, tag="qk")
                for m in range(4):
                    pst = [ps.tile([128, NF], f32, tag="ps", name="ps") for _ in range(N_CH)]
                    for ci in range(CH_T):
                        lhsT = mmv(wqk_sb[:, ci, m * 128 : (m + 1) * 128])
                        for n in range(N_CH):
                            nc.tensor.matmul(
                                pst[n][:],
                                lhsT,
                                mmv(x_cmp[:, ci, ns(n)]),
                                start=(ci == 0),
                                stop=(ci == CH_T - 1),
                            )
                    for n in range(N_CH):
                        nc.vector.tensor_scalar_add(
                            qk_sb[:, m, ns(n)], pst[n][:], bqk_sb[:, m : m + 1]
                        )

                # ---- U^T projection: uT[HW, o] = sum_ci x[ci, HW] wuT[ci, o] ----
                uT_sb = up.tile([128, HW_T, C], cdt, tag="uT")
                for m in range(HW_T):
                    pst = ps.tile([128, C], f32, tag="ps")
                    for ci in range(CH_T):
                        nc.tensor.matmul(
                            pst[:],
                            mmv(x_cmp[:, ci, m * 128 : (m + 1) * 128]),
                            mmv(wu_sb[:, ci, :]),
                            start=(ci == 0),
                            stop=(ci == CH_T - 1),
                        )
                    nc.vector.tensor_copy(uT_sb[:, m, :], pst[:])

                # ---- scores^T + exp: E[HW, hw] = exp(sum_c k[c,HW] q[c,hw] / 16) ----
                e_sb = ep.tile([128, HW_T, HW], cdt, tag="e")
                for m in range(HW_T):
                    pst = [ps.tile([128, NF], f32, tag="ps", name="ps") for _ in range(N_CH)]
                    for ci in range(CH_T):
                        lhsT = mmv(qk_sb[:, 2 + ci, m * 128 : (m + 1) * 128])
                        for n in range(N_CH):
                            nc.tensor.matmul(
                                pst[n][:],
                                lhsT,
                                mmv(qk_sb[:, ci, ns(n)]),
                                start=(ci == 0),
                                stop=(ci == CH_T - 1),
                            )
                    for n in range(N_CH):
                        nc.scalar.activation(
                            e_sb[:, m, ns(n)],
                            pst[n][:],
                            Act.Exp,
                            scale=1.0 / math.sqrt(C),
                        )

                # ---- Z[hw] = sum_HW E[HW, hw] via ones-matmul; R = 1/Z ----
                z_ps = zp.tile([1, HW], f32, tag="z")
                for k in range(HW_T):
                    lhsT = mmv(ones_col[:])
                    for n in range(N_CH):
                        nc.tensor.matmul(
                            z_ps[:, ns(n)],
                            lhsT,
                            mmv(e_sb[:, k, ns(n)]),
                            start=(k == 0),
                            stop=(k == HW_T - 1),
                        )
                # Reciprocal of a [1, 1024] row runs on a single DVE lane
                # (~6.5us); bounce Z through DRAM reshaped to [128, 8] so all
                # lanes work, then broadcast R back from DRAM (SBUF APs must
                # have nonzero partition step; DRAM sources are unconstrained).
                z_sb = rp.tile([1, HW], f32, tag="z_sb")
                nc.scalar.copy(z_sb[:], z_ps[:])
                z_dram = rd.tile([1, HW], f32, tag="zdram")
                nc.sync.dma_start(out=z_dram[:], in_=z_sb[:])
                z128 = rp.tile([128, HW // 128], f32, tag="z128")
                z_ap = z_dram[:]
                z_rs = bass.AP(
                    tensor=z_ap.tensor,
                    offset=z_ap.offset,
                    ap=[[HW // 128, 128], [1, HW // 128]],
                )
                nc.sync.dma_start(out=z128[:], in_=z_rs)
                r128 = rp.tile([128, HW // 128], f32, tag="r128")
                nc.vector.reciprocal(r128[:], z128[:])
                r_dram = rd.tile([1, HW], f32, tag="rdram")
                r_ap = r_dram[:]
                r_rs = bass.AP(
                    tensor=r_ap.tensor,
                    offset=r_ap.offset,
                    ap=[[HW // 128, 128], [1, HW // 128]],
                )
                nc.sync.dma_start(out=r_rs, in_=r128[:])
                rb_sb = rp.tile([128, HW], f32, tag="rb")
                r_bc = bass.AP(
                    tensor=r_ap.tensor,
                    offset=r_ap.offset,
                    ap=[[0, 128], [1, HW]],
                )
                nc.gpsimd.dma_start(out=rb_sb[:], in_=r_bc)

                def rbv(n):
                    return rb_sb[:, ns(n)]

                # ---- F[o, hw] = sum_HW uT[HW, o] E[HW, hw]; evict with norm+skip ----
                o_sb = op_.tile([128, CH_T, HW], f32, tag="o")
                for m in range(CH_T):
                    pst = [ps.tile([128, NF], f32, tag="ps", name="ps") for _ in range(N_CH)]
                    for k in range(HW_T):
                        lhsT = mmv(uT_sb[:, k, m * 128 : (m + 1) * 128])
                        for n in range(N_CH):
                            nc.tensor.matmul(
                                pst[n][:],
                                lhsT,
                                mmv(e_sb[:, k, ns(n)]),
                                start=(k == 0),
                                stop=(k == HW_T - 1),
                            )
                    for n in range(N_CH):
                        t_sb = tp.tile([128, NF], f32, tag="t")
                        nc.vector.tensor_mul(t_sb[:], pst[n][:], rbv(n))
                        nc.vector.scalar_tensor_tensor(
                            o_sb[:, m, ns(n)],
                            t_sb[:],
                            bf_sb[:, m : m + 1],
                            x_sb[:, m, ns(n)],
                            op0=Alu.add,
                            op1=Alu.add,
                        )
                for ci in range(CH_T):
                    nc.sync.dma_start(
                        out=out_d[b, ci * 128 : (ci + 1) * 128, :], in_=o_sb[:, ci, :]
                    )

    nc.compile()
    return nc


def kernel(x, w_in, b_in, w_out, b_out):
    global last_results
    _ensure_path()
    from concourse import bass_utils

    mm_dt = os.environ.get("ATTN_MM_DT", "fp32")
    bcast = os.environ.get("ATTN_BCAST", "dma")
    trace = os.environ.get("ATTN_TRACE", "0") == "1"

    key = (mm_dt, bcast)
    if key not in _cache:
        _cache[key] = _build(mm_dt, bcast)
    nc = _cache[key]

    x = np.ascontiguousarray(np.asarray(x, dtype=np.float32))
    w_in = np.asarray(w_in, dtype=np.float32)
    b_in = np.asarray(b_in, dtype=np.float32)
    w_out = np.asarray(w_out, dtype=np.float32)
    b_out = np.asarray(b_out, dtype=np.float32)

    # host-side weight prep (tiny)
    w_qk = w_in[: 2 * C]  # [512, 256]
    w_v = w_in[2 * C :]  # [256, 256]
    b_qk = b_in[: 2 * C]
    b_v = b_in[2 * C :]
    wqkT = np.ascontiguousarray(w_qk.T)  # [256, 512]
    w_u = w_out @ w_v  # fold output projection into V
    wuT = np.ascontiguousarray(w_u.T)  # [256, 256]
    bqk_t = np.ascontiguousarray(b_qk.reshape(4, 128).T)  # [128, 4]
    b_f = w_out @ b_v + b_out  # [256]
    bf_t = np.ascontiguousarray(b_f.reshape(2, 128).T)  # [128, 2]

    xr = x.reshape(B, C, HW)
    in_maps = []
    for c in range(NCORES):
        in_maps.append(
            {
                "x": np.ascontiguousarray(xr[c * BPC : (c + 1) * BPC]),
                "wqkT": wqkT,
                "wuT": wuT,
                "bqk": bqk_t,
                "bf": bf_t,
            }
        )

    res = bass_utils.run_bass_kernel_spmd(
        nc, in_maps, core_ids=list(range(NCORES)), trace=trace
    )
    last_results = res

    out = np.concatenate([res.results[i]["out"] for i in range(NCORES)], axis=0)
    return out.reshape(B, C, 32, 32).astype(np.float32)


# revision 9
# speedup vs baseline: 1.2230x; 1.2230x over previous
"""Trainium2 Bass kernel for nn_AttentionBlock (B=32, C=256, H=W=32).

Computation (per batch element b):
    qkv   = w_in @ x[b] + b_in          # 1x1 conv == channel matmul
    q,k,v = split(qkv)
    S^T   = scores stored transposed: S^T[HW, hw] = sum_c k[c,HW] q[c,hw]
    E     = exp(S^T / 16)               # softmax without max-subtraction (safe range)
    Z[hw] = sum_HW E[HW, hw]            # via ones-vector matmul (partition reduction)
    U^T   = x[b]^T @ (w_out @ w_v)^T    # w_out folded into V-projection (attention is
                                        # linear in V), computed directly transposed
    F     = F[o, hw] = sum_HW U^T[HW, o] E[HW, hw]
    out   = F * (1/Z) + (w_out @ b_v + b_out) + x[b]

Sharding: data-parallel over batch, 4 batch elements per core, 8 cores,
weights replicated. All matmul layouts chosen so no on-device transpose
is ever needed. Matmul operands are host-cast to the compute dtype.

Env knobs:
    ATTN_MM_DT   = fp32 | bf16          (matmul operand dtype; default bf16)
    ATTN_TRACE   = 0 | 1                (collect NTFF trace via run_bass_kernel_spmd)
"""

import math
import os
import sys

import numpy as np

B, C, HW = 32, 256, 1024
NCORES = 8
BPC = B // NCORES  # batch elements per core
CH_T = C // 128  # channel partition tiles (2)
HW_T = HW // 128  # spatial partition tiles (8)
NF = 512  # matmul free-dim chunk (one PSUM bank of fp32)
N_CH = HW // NF  # free-dim chunks (2)

_cache = {}
last_results = None  # BassKernelResults of the most recent run (for test.py)


def _ensure_path():
    for p in ("/opt/trn_rl_repo",):
        if os.path.isdir(p) and p not in sys.path:
            sys.path.append(p)


def _build(mm_dt: str):
    """Build + compile the Bass kernel once. Returns the compiled nc."""
    _ensure_path()
    import concourse.bass as bass
    import concourse.mybir as mybir
    import concourse.tile as tile
    from concourse import bacc

    f32 = mybir.dt.float32
    bf16 = mybir.dt.bfloat16
    Alu = mybir.AluOpType
    Act = mybir.ActivationFunctionType

    cdt = bf16 if mm_dt == "bf16" else f32  # storage dtype of matmul operands

    nc = bacc.Bacc(
        "TRN2", target_bir_lowering=False, debug=False, enable_asserts=False
    )

    x_d = nc.dram_tensor("x", [BPC, C, HW], f32, kind="ExternalInput")
    if mm_dt == "bf16":
        xm_d = nc.dram_tensor("xm", [BPC, C, HW], cdt, kind="ExternalInput")
    else:
        xm_d = x_d
    wqk_d = nc.dram_tensor("wqkT", [C, 2 * C], cdt, kind="ExternalInput")
    wu_d = nc.dram_tensor("wuT", [C, C], cdt, kind="ExternalInput")
    bias_d = nc.dram_tensor("bias", [128, 6], f32, kind="ExternalInput")
    out_d = nc.dram_tensor("out", [BPC, C, HW], f32, kind="ExternalOutput")

    def ns(n):
        return slice(n * NF, (n + 1) * NF)

    with tile.TileContext(nc) as tc:
        with (
            tc.tile_pool(name="const", bufs=1) as const,
            tc.tile_pool(name="xp", bufs=2) as xp,
            tc.tile_pool(name="qkp", bufs=2) as qkp,
            tc.tile_pool(name="up", bufs=2) as up,
            tc.tile_pool(name="ep", bufs=2) as ep,
            tc.tile_pool(name="rp", bufs=2) as rp,
            tc.tile_pool(name="op", bufs=2) as op_,
            tc.tile_pool(name="tp", bufs=4) as tp,
            tc.tile_pool(name="ps", bufs=6, space="PSUM") as ps,
            tc.tile_pool(name="zp", bufs=1, space="PSUM") as zp,
            tc.tile_pool(name="rd", bufs=2, space="DRAM") as rd,
        ):

            def load_x(b):
                # f32 copy for the residual (sync queue), compute-dtype copy
                # for matmul operands (gpsimd queue) — one DMA per ci tile.
                x_sb = xp.tile([128, CH_T, HW], f32, tag="x", name="x_sb")
                for ci in range(CH_T):
                    nc.sync.dma_start(
                        out=x_sb[:, ci, :], in_=x_d[b, ci * 128 : (ci + 1) * 128, :]
                    )
                if mm_dt == "bf16":
                    x_mm = xp.tile([128, CH_T, HW], cdt, tag="xmm", name="x_mm")
                    for ci in range(CH_T):
                        nc.gpsimd.dma_start(
                            out=x_mm[:, ci, :],
                            in_=xm_d[b, ci * 128 : (ci + 1) * 128, :],
                        )
                else:
                    x_mm = x_sb
                return x_sb, x_mm

            x_next = load_x(0)

            # ---- weights / constants (loaded once, single DMA each) ----
            wqk_sb = const.tile([128, CH_T, 2 * C], cdt, tag="wqk")
            nc.sync.dma_start(
                out=wqk_sb[:], in_=wqk_d[:].rearrange("(t p) f -> p t f", p=128)
            )
            wu_sb = const.tile([128, CH_T, C], cdt, tag="wu")
            nc.sync.dma_start(
                out=wu_sb[:], in_=wu_d[:].rearrange("(t p) f -> p t f", p=128)
            )
            bias_sb = const.tile([128, 6], f32, tag="bias")
            nc.sync.dma_start(out=bias_sb[:], in_=bias_d[:])
            bqk_sb = bias_sb[:, 0:4]
            bf_sb = bias_sb[:, 4:6]
            ones_col = const.tile([128, 1], cdt, tag="ones")
            nc.vector.memset(ones_col[:], 1.0)

            for b in range(BPC):
                x_sb, x_mm = x_next
                if b + 1 < BPC:
                    x_next = load_x(b + 1)

                # ---- Q,K projection: qk[m, hw], m = (q0,q1,k0,k1) ----
                qk_sb = qkp.tile([128, 4, HW], cdt, tag="qk")
                for m in range(4):
                    pst = [
                        ps.tile([128, NF], f32, tag="ps", name="ps")
                        for _ in range(N_CH)
                    ]
                    for ci in range(CH_T):
                        lhsT = wqk_sb[:, ci, m * 128 : (m + 1) * 128]
                        for n in range(N_CH):
                            nc.tensor.matmul(
                                pst[n][:],
                                lhsT,
                                x_mm[:, ci, ns(n)],
                                start=(ci == 0),
                                stop=(ci == CH_T - 1),
                            )
                    for n in range(N_CH):
                        nc.vector.tensor_scalar_add(
                            qk_sb[:, m, ns(n)], pst[n][:], bqk_sb[:, m : m + 1]
                        )

                # ---- U^T projection: uT[HW, o] = sum_ci x[ci, HW] wuT[ci, o] ----
                uT_sb = up.tile([128, HW_T, C], cdt, tag="uT")
                for m in range(HW_T):
                    pst = ps.tile([128, C], f32, tag="ps")
                    for ci in range(CH_T):
                        nc.tensor.matmul(
                            pst[:],
                            x_mm[:, ci, m * 128 : (m + 1) * 128],
                            wu_sb[:, ci, :],
                            start=(ci == 0),
                            stop=(ci == CH_T - 1),
                        )
                    nc.scalar.copy(uT_sb[:, m, :], pst[:])

                # ---- scores^T + exp: E[HW, hw] = exp(sum_c k[c,HW] q[c,hw]/16) ----
                e_sb = ep.tile([128, HW_T, HW], cdt, tag="e")
                for m in range(HW_T):
                    pst = [
                        ps.tile([128, NF], f32, tag="ps", name="ps")
                        for _ in range(N_CH)
                    ]
                    for ci in range(CH_T):
                        lhsT = qk_sb[:, 2 + ci, m * 128 : (m + 1) * 128]
                        for n in range(N_CH):
                            nc.tensor.matmul(
                                pst[n][:],
                                lhsT,
                                qk_sb[:, ci, ns(n)],
                                start=(ci == 0),
                                stop=(ci == CH_T - 1),
                            )
                    for n in range(N_CH):
                        nc.scalar.activation(
                            e_sb[:, m, ns(n)],
                            pst[n][:],
                            Act.Exp,
                            scale=1.0 / math.sqrt(C),
                        )

                # ---- Z[hw] = sum_HW E[HW, hw] via ones-matmul; R = 1/Z ----
                z_ps = zp.tile([1, HW], f32, tag="z")
                for k in range(HW_T):
                    lhsT = ones_col[:]
                    for n in range(N_CH):
                        nc.tensor.matmul(
                            z_ps[:, ns(n)],
                            lhsT,
                            e_sb[:, k, ns(n)],
                            start=(k == 0),
                            stop=(k == HW_T - 1),
                        )
                # Reciprocal of a [1, 1024] row runs on a single DVE lane
                # (~6.5us); bounce Z through DRAM reshaped to [128, 8] so all
                # lanes work, then broadcast R back from DRAM (SBUF APs must
                # have nonzero partition step; DRAM sources are unconstrained).
                z_sb = rp.tile([1, HW], f32, tag="z_sb")
                nc.scalar.copy(z_sb[:], z_ps[:])
                z_dram = rd.tile([1, HW], f32, tag="zdram")
                nc.scalar.dma_start(out=z_dram[:], in_=z_sb[:])
                z128 = rp.tile([128, HW // 128], f32, tag="z128")
                z_ap = z_dram[:]
                z_rs = bass.AP(
                    tensor=z_ap.tensor,
                    offset=z_ap.offset,
                    ap=[[HW // 128, 128], [1, HW // 128]],
                )
                nc.scalar.dma_start(out=z128[:], in_=z_rs)
                r128 = rp.tile([128, HW // 128], f32, tag="r128")
                nc.vector.reciprocal(r128[:], z128[:])
                r_dram = rd.tile([1, HW], f32, tag="rdram")
                r_ap = r_dram[:]
                r_rs = bass.AP(
                    tensor=r_ap.tensor,
                    offset=r_ap.offset,
                    ap=[[HW // 128, 128], [1, HW // 128]],
                )
                nc.gpsimd.dma_start(out=r_rs, in_=r128[:])
                rb_sb = rp.tile([128, HW], f32, tag="rb")
                r_bc = bass.AP(
                    tensor=r_ap.tensor,
                    offset=r_ap.offset,
                    ap=[[0, 128], [1, HW]],
                )
                nc.gpsimd.dma_start(out=rb_sb[:], in_=r_bc)

                # ---- F[o,hw] = sum_HW uT[HW,o] E[HW,hw]; evict norm+bias+skip ----
                o_sb = op_.tile([128, CH_T, HW], f32, tag="o")
                for m in range(CH_T):
                    pst = [
                        ps.tile([128, NF], f32, tag="ps", name="ps")
                        for _ in range(N_CH)
                    ]
                    for k in range(HW_T):
                        lhsT = uT_sb[:, k, m * 128 : (m + 1) * 128]
                        for n in range(N_CH):
                            nc.tensor.matmul(
                                pst[n][:],
                                lhsT,
                                e_sb[:, k, ns(n)],
                                start=(k == 0),
                                stop=(k == HW_T - 1),
                            )
                    for n in range(N_CH):
                        t_sb = tp.tile([128, NF], f32, tag="t", name="t_sb")
                        nc.vector.tensor_mul(t_sb[:], pst[n][:], rb_sb[:, ns(n)])
                        nc.vector.scalar_tensor_tensor(
                            o_sb[:, m, ns(n)],
                            t_sb[:],
                            bf_sb[:, m : m + 1],
                            x_sb[:, m, ns(n)],
                            op0=Alu.add,
                            op1=Alu.add,
                        )
                for ci in range(CH_T):
                    nc.sync.dma_start(
                        out=out_d[b, ci * 128 : (ci + 1) * 128, :], in_=o_sb[:, ci, :]
                    )

    nc.compile()
    return nc


def kernel(x, w_in, b_in, w_out, b_out):
    global last_results
    _ensure_path()
    import ml_dtypes
    from concourse import bass_utils

    mm_dt = os.environ.get("ATTN_MM_DT", "bf16")
    trace = os.environ.get("ATTN_TRACE", "0") == "1"

    if mm_dt not in _cache:
        _cache[mm_dt] = _build(mm_dt)
    nc = _cache[mm_dt]

    np_cdt = ml_dtypes.bfloat16 if mm_dt == "bf16" else np.float32

    x = np.ascontiguousarray(np.asarray(x, dtype=np.float32))
    w_in = np.asarray(w_in, dtype=np.float32)
    b_in = np.asarray(b_in, dtype=np.float32)
    w_out = np.asarray(w_out, dtype=np.float32)
    b_out = np.asarray(b_out, dtype=np.float32)

    # host-side weight prep (tiny)
    w_qk = w_in[: 2 * C]  # [512, 256]
    w_v = w_in[2 * C :]  # [256, 256]
    b_qk = b_in[: 2 * C]
    b_v = b_in[2 * C :]
    wqkT = np.ascontiguousarray(w_qk.T.astype(np_cdt))  # [256, 512]
    w_u = w_out @ w_v  # fold output projection into V
    wuT = np.ascontiguousarray(w_u.T.astype(np_cdt))  # [256, 256]
    b_f = w_out @ b_v + b_out  # [256]
    bias = np.concatenate(
        [b_qk.reshape(4, 128).T, b_f.reshape(2, 128).T], axis=1
    )  # [128, 6]
    bias = np.ascontiguousarray(bias.astype(np.float32))

    xr = x.reshape(B, C, HW)
    xm = xr.astype(np_cdt) if mm_dt == "bf16" else None
    in_maps = []
    for c in range(NCORES):
        m = {
            "x": np.ascontiguousarray(xr[c * BPC : (c + 1) * BPC]),
            "wqkT": wqkT,
            "wuT": wuT,
            "bias": bias,
        }
        if mm_dt == "bf16":
            m["xm"] = np.ascontiguousarray(xm[c * BPC : (c + 1) * BPC])
        in_maps.append(m)

    res = bass_utils.run_bass_kernel_spmd(
        nc, in_maps, core_ids=list(range(NCORES)), trace=trace
    )
    last_results = res

    out = np.concatenate([res.results[i]["out"] for i in range(NCORES)], axis=0)
    return out.reshape(B, C, 32, 32).astype(np.float32)
